# revision 1
# baseline (speedup 1.0000x reference)
"""Causal GQA multi-head attention (RMSNorm-QK + RoPE) on 8 Trainium2 cores.

Sharding: tensor-parallel over heads. Core c owns Q heads {2c, 2c+1} and KV
head c//2 (GQA group). Each core computes its heads' attention end-to-end and
a partial output projection (row-sharded Wo); the host sums the 8 partials.

Device layout strategy (per core):
  - projections produce qT/kT/vT in [dh(part), token(free)] layout so that
    QK^T needs no transposes at all (scoresT blocks [keys, q]);
  - softmax runs WITHOUT max-subtraction (RMS-normed scores are O(+-6), exp
    is safe in fp32) so no cross-partition max is needed;
  - prob row-sums come from an accumulated ones-matmul on the PE;
  - RMSNorm is folded: gamma into the PSUM-eviction scale, the q-side
    1/sqrt(sumsq) (which also absorbs the 1/sqrt(dh) score scale) into a
    broadcast multiply after RoPE, and the k-side rsqrt(var+eps) into the
    per-partition scale of the exp() activation.
"""

import sys

sys.path.insert(0, "/opt/trn_rl_repo")

from contextlib import ExitStack

import ml_dtypes
import numpy as np

import concourse.bass as bass
import concourse.tile as tile
from concourse import bacc, mybir
from concourse.bass_utils import run_bass_kernel_spmd
from concourse.masks import make_identity

B, S, D = 2, 2048, 2048
H, HKV, DH = 16, 4, 128
T = B * S  # 4096 tokens
P = 128
NCORES = 8
HPC = H // NCORES  # 2 q heads per core
EPS = 1e-6
ROPE_THETA = 10000.0
BF = mybir.dt.bfloat16
F32 = mybir.dt.float32
BFNP = ml_dtypes.bfloat16

Copy = mybir.ActivationFunctionType.Copy
Exp = mybir.ActivationFunctionType.Exp
Sqrt = mybir.ActivationFunctionType.Sqrt
Square = mybir.ActivationFunctionType.Square
MULT = mybir.AluOpType.mult
ADD = mybir.AluOpType.add


def _body(ctx: ExitStack, tc: tile.TileContext, xt, wqkv, wo, cos, sin, masks, gq, gk, out):
    nc = tc.nc

    const = ctx.enter_context(tc.tile_pool(name="const", bufs=1))
    res = ctx.enter_context(tc.tile_pool(name="res", bufs=1))
    xt_pool = ctx.enter_context(tc.tile_pool(name="xtp", bufs=24))
    sq_pool = ctx.enter_context(tc.tile_pool(name="sqp", bufs=4))
    exp_pool = ctx.enter_context(tc.tile_pool(name="exq", bufs=8))
    rope_pool = ctx.enter_context(tc.tile_pool(name="rop", bufs=3))
    rsb_pool = ctx.enter_context(tc.tile_pool(name="rsb", bufs=4))
    row_pool = ctx.enter_context(tc.tile_pool(name="row", bufs=4))
    out_pool = ctx.enter_context(tc.tile_pool(name="outp", bufs=4))
    attu_pool = ctx.enter_context(tc.tile_pool(name="attu", bufs=4))
    # PSUM: 2 wide (2-bank) slots + 4 narrow banks = 8 banks total
    psum_w = ctx.enter_context(tc.tile_pool(name="psw", bufs=2, space="PSUM"))
    psum_n = ctx.enter_context(tc.tile_pool(name="psn", bufs=4, space="PSUM"))
    dram = ctx.enter_context(tc.tile_pool(name="drm", bufs=1, space="DRAM"))

    # ---- constants / resident weights ----
    ones_bf = const.tile([P, 1], BF, name="ones", tag="ones")
    nc.vector.memset(ones_bf[:], 1.0)
    ident = const.tile([P, P], BF, name="ident", tag="ident")
    make_identity(nc, ident[:])
    epsq_t = const.tile([P, 1], F32, name="epsq", tag="epsq")
    nc.vector.memset(epsq_t[:], P * EPS)
    epsk_t = const.tile([P, 1], F32, name="epsk", tag="epsk")
    nc.vector.memset(epsk_t[:], EPS)
    gq_t = const.tile([P, 1], F32, name="gq", tag="gq")
    gk_t = const.tile([P, 1], F32, name="gk", tag="gk")
    cos_t = const.tile([P, T], BF, name="cos", tag="cos")
    sin_t = const.tile([P, T], BF, name="sin", tag="sin")
    mask_t = []
    for i in range(4):
        m = const.tile([P, 512], BF, name=f"mask{i}", tag=f"mask{i}")
        mask_t.append(m)
    wqkv_sb = []
    for k in range(16):
        w = const.tile([P, 512], BF, name=f"wqkv{k}", tag=f"wqkv{k}")
        nc.sync.dma_start(w[:], wqkv[k])
        wqkv_sb.append(w)
    wo_sb = []
    for h in range(HPC):
        w = const.tile([P, D], BF, name=f"wo{h}", tag=f"wo{h}")
        wo_sb.append(w)

    def load_deferred_consts():
        nc.sync.dma_start(gq_t[:], gq[:])
        nc.sync.dma_start(gk_t[:], gk[:])
        nc.sync.dma_start(cos_t[:], cos[:])
        nc.sync.dma_start(sin_t[:], sin[:])
        for i in range(4):
            nc.sync.dma_start(mask_t[i][:], masks[i])
        for h in range(HPC):
            nc.sync.dma_start(wo_sb[h][:], wo[h])

    # resident activations: [dh, token] layouts
    qk_t = [res.tile([P, T], BF, name=f"qT{h}", tag=f"qT{h}") for h in range(HPC)]  # q heads
    kT = res.tile([P, T], BF, name="kT", tag="kT")
    vT_sb = res.tile([P, T], BF, name="vT", tag="vT")
    v_kd = res.tile([P, T], BF, name="vkd", tag="vkd")  # v as [keys(part), dh] chunks
    rs_k_col = res.tile([P, T // P], F32, name="rskc", tag="rskc")
    att = [res.tile([P, S], BF, name=f"att{h}", tag=f"att{h}") for h in range(HPC)]

    # ---- phase 1: fused qkv projection + per-tile rmsnorm/rope epilogue ----
    def rope_tile(dst_slice, src_slice, cols, rsb_ap):
        """dst = (src*cos + rot(src)*sin) [* rsb]; src is a [P,512] bf16 slice."""
        t1 = rope_pool.tile([P, 512], F32, name="t1", tag="t1")
        t2 = rope_pool.tile([P, 512], F32, name="t2", tag="t2")
        # rot(x)[0:64] = -x[64:128]; rot(x)[64:128] = x[0:64]
        nc.vector.tensor_scalar_mul(t2[0:64, :], src_slice[64:128, :], -1.0)
        nc.vector.tensor_copy(t2[64:128, :], src_slice[0:64, :])
        nc.vector.tensor_tensor(t1[:], src_slice[:], cos_t[:, cols], MULT)
        nc.vector.tensor_tensor(t2[:], t2[:], sin_t[:, cols], MULT)
        if rsb_ap is None:
            nc.vector.tensor_tensor(dst_slice[:], t1[:], t2[:], ADD)
        else:
            nc.vector.tensor_tensor(t1[:], t1[:], t2[:], ADD)
            nc.vector.tensor_tensor(dst_slice[:], t1[:], rsb_ap[:], MULT)

    sc = dram.tile([1, T], F32, name="scratch", tag="scratch")

    def proj_nb(nb):
        xts = []
        for k in range(16):
            xtile = xt_pool.tile([P, 1024], BF, name="xtt", tag="xtt")
            nc.sync.dma_start(xtile[:], xt[k, :, nb * 1024:(nb + 1) * 1024])
            xts.append(xtile)
        if nb == 0:
            load_deferred_consts()
        psms = []
        for m in range(4):
            pss = psum_w.tile([P, 1024], F32, name="psw", tag="psw")
            psms.append(pss)
            for k in range(16):
                for n2 in range(2):
                    nc.tensor.matmul(
                        pss[:, n2 * 512:(n2 + 1) * 512],
                        wqkv_sb[k][:, m * 128:(m + 1) * 128],
                        xts[k][:, n2 * 512:(n2 + 1) * 512],
                        start=(k == 0),
                        stop=(k == 15),
                        skip_group_check=True,
                    )
        for m in range(4):
            pss = psms[m]
            wcols = slice(nb * 1024, (nb + 1) * 1024)
            if m < 2:  # q heads
                nc.scalar.activation(qk_t[m][:, wcols], pss[:], Copy, bias=0.0, scale=gq_t[:])
            elif m == 2:
                nc.scalar.activation(kT[:, wcols], pss[:], Copy, bias=0.0, scale=gk_t[:])
            else:
                nc.scalar.activation(vT_sb[:, wcols], pss[:], Copy)
                continue
            for n2 in range(2):
                col0 = nb * 1024 + n2 * 512
                cols = slice(col0, col0 + 512)
                sq = sq_pool.tile([P, 512], BF, name="sq", tag="sq")
                nc.scalar.activation(sq[:], pss[:, n2 * 512:(n2 + 1) * 512], Square)
                ps_ss = psum_n.tile([P, 512], F32, name="pss", tag="psn")
                nc.tensor.matmul(ps_ss[:1, :], ones_bf[:], sq[:], start=True, stop=True)
                sd = row_pool.tile([1, 512], F32, name="row", tag="row")
                if m < 2:
                    # rs_q = 1/sqrt(sumsq + 128*eps) == rsqrt(var+eps)/sqrt(128)
                    nc.scalar.activation(sd[:], ps_ss[:1, :], Sqrt, bias=epsq_t[:1, :])
                    rsq = row_pool.tile([1, 512], F32, name="row", tag="row")
                    nc.vector.reciprocal_approx_fast(rsq[:], sd[:])
                    rsb = rsb_pool.tile([P, 512], F32, name="rsb", tag="rsb")
                    nc.gpsimd.partition_broadcast(rsb[:], rsq[:])
                    rope_tile(qk_t[m][:, cols], qk_t[m][:, cols], cols, rsb)
                else:
                    # rs_k = rsqrt(var + eps)
                    nc.scalar.activation(sd[:], ps_ss[:1, :], Sqrt, bias=epsk_t[:1, :], scale=1.0 / P)
                    rkr = row_pool.tile([1, 512], F32, name="row", tag="row")
                    nc.vector.reciprocal_approx_fast(rkr[:], sd[:])
                    nc.sync.dma_start(sc[0, col0:col0 + 512], rkr[:])
                    # immediately resolve this tile's 4 key-chunk columns
                    ch0 = col0 // P
                    nc.sync.dma_start(
                        rs_k_col[:, ch0:ch0 + 4],
                        sc[0:1, col0:col0 + 512].rearrange("a (c p) -> p (a c)", p=P),
                    )
                    rope_tile(kT[:, cols], kT[:, cols], cols, None)

        # v transposes for this nb's key chunks (spread through proj phase)
        for g2 in range(2):
            pst = psum_n.tile([P, 512], BF, name="pst", tag="psn")
            for c4 in range(4):
                c = nb * 8 + g2 * 4 + c4
                nc.tensor.transpose(pst[:, c4 * P:(c4 + 1) * P], vT_sb[:, c * P:(c + 1) * P], ident[:])
            nc.scalar.copy(v_kd[:, (nb * 2 + g2) * 512:(nb * 2 + g2 + 1) * 512], pst[:])

    # ---- phase 2: attention (per batch, per local q head) + Wo partial ----
    def attn_wo_batch(b):
        tb = b * S
        for h in range(HPC):
            for qt in range(4):
                qs = tb + qt * 512
                ps_att = psum_n.tile([P, 512], F32, name="psA", tag="psn")
                ps_sum = psum_n.tile([P, 512], F32, name="psB", tag="psn")
                nkc = 4 * qt + 4
                for kc in range(nkc):
                    ks = tb + kc * P
                    gkc = 16 * b + kc  # global key chunk
                    off = max(0, P * kc - 512 * qt)  # causal: queries >= key start
                    if kc % 2 == 0:
                        ps_s = psum_n.tile([P, 512], F32, name="psS", tag="psn")
                    else:
                        ps_s = psum_w.tile([P, 512], F32, name="psSw", tag="psw")
                    nc.tensor.matmul(
                        ps_s[:, off:], kT[:, ks:ks + P], qk_t[h][:, qs + off:qs + 512],
                        start=True, stop=True, skip_group_check=True,
                    )
                    ex = exp_pool.tile([P, 512], BF, name="ex", tag="ex")
                    nc.scalar.activation(
                        ex[:, off:], ps_s[:, off:], Exp, scale=rs_k_col[:, gkc:gkc + 1],
                    )
                    if kc >= 4 * qt:
                        moff = kc - 4 * qt
                        nc.vector.tensor_tensor(
                            ex[:, off:], ex[:, off:], mask_t[moff][:, off:], MULT
                        )
                    nc.tensor.matmul(
                        ps_att[:, off:], v_kd[:, gkc * P:(gkc + 1) * P], ex[:, off:],
                        start=(kc == 0), stop=(kc == nkc - 1), skip_group_check=True,
                    )
                    nc.tensor.matmul(
                        ps_sum[:1, off:], ones_bf[:], ex[:, off:],
                        start=(kc == 0), stop=(kc == nkc - 1), skip_group_check=True,
                    )
                att_un = attu_pool.tile([P, 512], F32, name="attu", tag="attu")
                nc.vector.tensor_copy(att_un[:], ps_att[:])
                rrow = row_pool.tile([1, 512], F32, name="row", tag="row")
                nc.vector.reciprocal_approx_fast(rrow[:], ps_sum[:1, :])
                rsb = rsb_pool.tile([P, 512], F32, name="rsb", tag="rsb")
                nc.gpsimd.partition_broadcast(rsb[:], rrow[:])
                nc.vector.tensor_tensor(att[h][:, qt * 512:(qt + 1) * 512], att_un[:], rsb[:], MULT)
        # Wo partial for batch b: out[t, e] += att_h[:, t].T @ woT_h
        for tt in range(16):
            for et in range(2):
                pso = psum_w.tile([P, 1024], F32, name="pso", tag="psw")
                for h in range(HPC):
                    for n2 in range(2):
                        nc.tensor.matmul(
                            pso[:, n2 * 512:(n2 + 1) * 512],
                            att[h][:, tt * P:(tt + 1) * P],
                            wo_sb[h][:, et * 1024 + n2 * 512:et * 1024 + (n2 + 1) * 512],
                            start=(h == 0), stop=(h == HPC - 1), skip_group_check=True,
                        )
                osb = out_pool.tile([P, 1024], BF, name="osb", tag="osb")
                if (tt + et) % 2 == 0:
                    nc.vector.tensor_copy(osb[:], pso[:])
                else:
                    nc.scalar.copy(osb[:], pso[:])
                nc.sync.dma_start(
                    out[tb + tt * P: tb + (tt + 1) * P, et * 1024:(et + 1) * 1024], osb[:]
                )

    for nb in range(4):
        proj_nb(nb)
    for b in range(B):
        attn_wo_batch(b)


_NC_CACHE = None


def build_nc():
    global _NC_CACHE
    if _NC_CACHE is not None:
        return _NC_CACHE
    nc = bacc.Bacc(None, target_bir_lowering=False)
    xt = nc.dram_tensor("xt", [16, P, T], BF, kind="ExternalInput")
    wqkv = nc.dram_tensor("wqkv", [16, P, 512], BF, kind="ExternalInput")
    wo = nc.dram_tensor("wo", [HPC, P, D], BF, kind="ExternalInput")
    cos = nc.dram_tensor("cos", [P, T], BF, kind="ExternalInput")
    sin = nc.dram_tensor("sin", [P, T], BF, kind="ExternalInput")
    masks = nc.dram_tensor("masks", [4, P, 512], BF, kind="ExternalInput")
    gq = nc.dram_tensor("gq", [P, 1], F32, kind="ExternalInput")
    gk = nc.dram_tensor("gk", [P, 1], F32, kind="ExternalInput")
    out = nc.dram_tensor("out", [T, D], BF, kind="ExternalOutput")
    with tile.TileContext(nc) as tc:
        with ExitStack() as ctx:
            _body(ctx, tc, xt[:], wqkv[:], wo[:], cos[:], sin[:], masks[:], gq[:], gk[:], out[:])
    nc.compile()
    _NC_CACHE = nc
    return nc


def _host_tables():
    pos = np.arange(S, dtype=np.float64)
    inv_freq = 1.0 / (ROPE_THETA ** (np.arange(0, DH, 2, dtype=np.float64) / DH))
    ang = pos[:, None] * inv_freq[None, :]  # [S, 64]
    cos_s = np.concatenate([np.cos(ang), np.cos(ang)], axis=-1)  # [S, 128]
    sin_s = np.concatenate([np.sin(ang), np.sin(ang)], axis=-1)
    cos_full = np.tile(cos_s.T, (1, B)).astype(BFNP)  # [128, T] batch-tiled
    sin_full = np.tile(sin_s.T, (1, B)).astype(BFNP)
    j = np.arange(P)[None, :, None]
    i = np.arange(512)[None, None, :]
    m = np.arange(4)[:, None, None]
    masks = (i >= j + P * m).astype(BFNP)  # [4, 128, 512]
    return cos_full, sin_full, masks


def kernel(qkv, Wq, Wk, Wv, Wo, q_gamma, k_gamma):
    qkv = np.asarray(qkv, dtype=np.float32)
    Wq = np.asarray(Wq, dtype=np.float32)
    Wk = np.asarray(Wk, dtype=np.float32)
    Wv = np.asarray(Wv, dtype=np.float32)
    Wo = np.asarray(Wo, dtype=np.float32)
    q_gamma = np.asarray(q_gamma, dtype=np.float32)
    k_gamma = np.asarray(k_gamma, dtype=np.float32)

    nc = build_nc()
    cos_full, sin_full, masks = _host_tables()
    xt = np.ascontiguousarray(qkv.reshape(T, D).T).astype(BFNP).reshape(16, P, T)
    gq = np.ascontiguousarray(q_gamma.reshape(P, 1))
    gk = np.ascontiguousarray(k_gamma.reshape(P, 1))

    in_maps = []
    for c in range(NCORES):
        kv = c // 2
        wq_c = Wq[2 * c * DH:(2 * c + 2) * DH, :]  # [256, D]
        wk_c = Wk[kv * DH:(kv + 1) * DH, :]  # [128, D]
        wv_c = Wv[kv * DH:(kv + 1) * DH, :]
        wqkv_c = np.concatenate([wq_c, wk_c, wv_c], axis=0).T  # [D, 512]
        wqkv_c = np.ascontiguousarray(wqkv_c).astype(BFNP).reshape(16, P, 512)
        wo_c = np.stack(
            [np.ascontiguousarray(Wo[:, (2 * c + h) * DH:(2 * c + h + 1) * DH].T)
             for h in range(HPC)]
        ).astype(BFNP)  # [2, 128, D]
        in_maps.append({
            "xt": xt, "wqkv": wqkv_c, "wo": wo_c,
            "cos": cos_full, "sin": sin_full, "masks": masks,
            "gq": gq, "gk": gk,
        })

    res = run_bass_kernel_spmd(nc, in_maps, core_ids=list(range(NCORES)))
    acc = res.results[0]["out"].astype(np.float32)
    for c in range(1, NCORES):
        acc += res.results[c]["out"].astype(np.float32)
    return acc.reshape(B, S, D)



# revision 5
# speedup vs baseline: 1.0693x; 1.0693x over previous
"""Causal GQA multi-head attention (RMSNorm-QK + RoPE) on 8 Trainium2 cores.

Sharding: (batch, kv-group). Core c owns batch c//4 and GQA group c%4,
i.e. 4 q heads + 1 kv head for one batch of 2048 tokens. This splits the
total work exactly 8 ways with zero duplicated projection flops (the old
head-sharding computed each kv head twice and projected both batches on
every core). Each core emits a partial [S, D] output (row-sharded Wo);
the host sums the 4 partials per batch.

Per-core structure (all matmuls bf16, K=M=128, N<=512):
  - proj: x[D, S] @ wqkv -> qT (4 heads), kT, vT in [dh, token] layout.
    First block is emitted k-outer so the PE starts after ~2 DMA tiles.
  - rmsnorm: sumsq via ones-matmul; rsqrt as exp(-0.5*ln(x)) so the
    whole kernel uses ONE activation table set (natural_log_exp);
    q-side rs (absorbs 1/sqrt(dh)) and k-side rs are folded into the
    qT/kT tiles themselves during the RoPE epilogue, so attention exp
    needs no per-partition scale.
  - attention: scoresT blocks [keys, queries]; causal mask folded into
    the scores psum via a tiny [128,128] identity-matmul add of -30000
    on the diagonal block only; exp WITHOUT max-subtraction; rowsums
    via accumulated ones-matmul; the inner loop is software-pipelined
    (scores run LAG=2 chunks ahead of AV) so the PE never waits on the
    scalar engine's exp.
  - Wo: row-sharded partial, interleaved per query-block.
"""

import sys

sys.path.insert(0, "/opt/trn_rl_repo")

from contextlib import ExitStack

import ml_dtypes
import numpy as np

import concourse.bass as bass
import concourse.tile as tile
from concourse import bacc, mybir
from concourse.bass_utils import run_bass_kernel_spmd
from concourse.masks import make_identity

B, S, D = 2, 2048, 2048
H, HKV, DH = 16, 4, 128
P = 128
NCORES = 8
HPC = 4  # q heads per core
EPS = 1e-6
ROPE_THETA = 10000.0
BF = mybir.dt.bfloat16
F32 = mybir.dt.float32
BFNP = ml_dtypes.bfloat16

Copy = mybir.ActivationFunctionType.Copy
Exp = mybir.ActivationFunctionType.Exp
Ln = mybir.ActivationFunctionType.Ln
MULT = mybir.AluOpType.mult
ADD = mybir.AluOpType.add

NBLK = 4  # 512-token blocks
BLK = S // NBLK


def _body(ctx: ExitStack, tc: tile.TileContext, xt, wqkv, wo, cos, sins, masktri, gq, gk, out):
    nc = tc.nc

    const = ctx.enter_context(tc.tile_pool(name="const", bufs=1))
    res = ctx.enter_context(tc.tile_pool(name="res", bufs=1))
    xt_pool = ctx.enter_context(tc.tile_pool(name="xtp", bufs=2))
    sq_pool = ctx.enter_context(tc.tile_pool(name="sqp", bufs=3))
    row_pool = ctx.enter_context(tc.tile_pool(name="row", bufs=4))
    rsb_pool = ctx.enter_context(tc.tile_pool(name="rsb", bufs=3))
    rope_pool = ctx.enter_context(tc.tile_pool(name="rop", bufs=2))
    exp_pool = ctx.enter_context(tc.tile_pool(name="exq", bufs=6))
    attu_pool = ctx.enter_context(tc.tile_pool(name="attu", bufs=2))
    att_pool = ctx.enter_context(tc.tile_pool(name="attp", bufs=2))
    osb_pool = ctx.enter_context(tc.tile_pool(name="osb", bufs=2))
    # PSUM: 8 banks = sc(3) + attps(1) + sumps(1) + pp(3)
    scp = ctx.enter_context(tc.tile_pool(name="scp", bufs=3, space="PSUM"))
    attps = ctx.enter_context(tc.tile_pool(name="atps", bufs=1, space="PSUM"))
    sumps = ctx.enter_context(tc.tile_pool(name="smps", bufs=1, space="PSUM"))
    pp = ctx.enter_context(tc.tile_pool(name="pp", bufs=3, space="PSUM"))

    # ---- constants / resident weights ----
    ones_bf = const.tile([P, 1], BF, name="ones", tag="ones")
    nc.vector.memset(ones_bf[:], 1.0)
    ident = const.tile([P, P], BF, name="ident", tag="ident")
    make_identity(nc, ident[:])
    gq_t = const.tile([P, 1], F32, name="gq", tag="gq")
    gk_t = const.tile([P, 1], F32, name="gk", tag="gk")
    cos_t = const.tile([P, S], BF, name="cos", tag="cos")
    sins_t = const.tile([P, S], BF, name="sins", tag="sins")
    mask_t = const.tile([P, P], BF, name="mask", tag="mask")
    epsq_t = const.tile([1, 1], F32, name="epsq", tag="epsq")
    nc.vector.memset(epsq_t[:], P * EPS)
    epsk_t = const.tile([1, 1], F32, name="epsk", tag="epsk")
    nc.vector.memset(epsk_t[:], EPS)
    wqkv_sb = [const.tile([P, 768], BF, name=f"wqkv{k}", tag=f"wqkv{k}") for k in range(16)]
    wo_sb = [const.tile([P, D], BF, name=f"wo{h}", tag=f"wo{h}") for h in range(HPC)]

    # resident activations, [dh, token] layouts
    qT = [res.tile([P, S], BF, name=f"qT{h}", tag=f"qT{h}") for h in range(HPC)]
    kT = res.tile([P, S], BF, name="kT", tag="kT")
    vT = res.tile([P, S], BF, name="vT", tag="vT")
    v_kd = res.tile([P, S], BF, name="vkd", tag="vkd")  # [keys, dh] chunks

    xts = [[None] * 16 for _ in range(NBLK)]

    def dma_block(nb):
        for k in range(16):
            t = xt_pool.tile([P, BLK], BF, name=f"xt{k}", tag=f"xt{k}")
            nc.sync.dma_start(t[:], xt[k, :, nb * BLK:(nb + 1) * BLK])
            xts[nb][k] = t

    # preamble DMAs: tiny scales, then wqkv/x-block-0 interleaved so the
    # first matmul can start after ~2 tiles have landed.
    nc.sync.dma_start(gq_t[:], gq[:])
    nc.sync.dma_start(gk_t[:], gk[:])
    for k in range(16):
        nc.sync.dma_start(wqkv_sb[k][:], wqkv[k])
        t = xt_pool.tile([P, BLK], BF, name=f"xt{k}", tag=f"xt{k}")
        nc.sync.dma_start(t[:], xt[k, :, 0:BLK])
        xts[0][k] = t
    nc.sync.dma_start(cos_t[:], cos[:])
    nc.sync.dma_start(sins_t[:], sins[:])
    nc.sync.dma_start(mask_t[:], masktri[:])

    # ---- phase 1: fused qkv projection + rmsnorm/rope epilogue ----
    def rope_tile(dst, cols, rsb):
        """dst = (dst*cos + rot(dst)*sin) * rsb, in place; sins has the
        sign of the rotation baked into its first 64 rows."""
        t1 = rope_pool.tile([P, BLK], BF, name="t1", tag="t1")
        t2 = rope_pool.tile([P, BLK], BF, name="t2", tag="t2")
        nc.vector.tensor_copy(t2[0:64, :], dst[64:128, :])
        nc.vector.tensor_copy(t2[64:128, :], dst[0:64, :])
        nc.vector.tensor_tensor(t2[:], t2[:], sins_t[:, cols], MULT)
        nc.vector.tensor_tensor(t1[:], dst[:], cos_t[:, cols], MULT)
        nc.vector.tensor_tensor(t1[:], t1[:], t2[:], ADD)
        nc.vector.tensor_tensor(dst[:], t1[:], rsb[:], MULT)

    def epilogue(nb, m, ps):
        cols = slice(nb * BLK, (nb + 1) * BLK)
        if m == 5:  # v: evict + transpose to [keys, dh] chunks
            nc.vector.tensor_copy(vT[:, cols], ps[:])
            pst = pp.tile([P, BLK], BF, name="pst", tag="pp")
            for i in range(4):
                c = nb * 4 + i
                nc.tensor.transpose(pst[:, i * P:(i + 1) * P], vT[:, c * P:(c + 1) * P], ident[:])
            nc.scalar.copy(v_kd[:, cols], pst[:])
            return
        if m < 4:
            dst = qT[m]
            nc.scalar.activation(dst[:, cols], ps[:], Copy, bias=0.0, scale=gq_t[:])
        else:
            dst = kT
            nc.scalar.activation(dst[:, cols], ps[:], Copy, bias=0.0, scale=gk_t[:])
        sq = sq_pool.tile([P, BLK], BF, name="sq", tag="sq")
        nc.vector.tensor_tensor(sq[:], dst[:, cols], dst[:, cols], MULT)
        psr = pp.tile([P, BLK], F32, name="psr", tag="pp")
        nc.tensor.matmul(psr[:1, :], ones_bf[:], sq[:], start=True, stop=True,
                         skip_group_check=True)
        row = row_pool.tile([1, BLK], F32, name="row", tag="row")
        rs = row_pool.tile([1, BLK], F32, name="rs", tag="row")
        if m < 4:
            # rs_q = 1/sqrt(sumsq + 128*eps) == rsqrt(var+eps)/sqrt(dh)
            nc.scalar.activation(row[:], psr[:1, :], Ln, bias=epsq_t[:1, :])
        else:
            # rs_k = rsqrt(var + eps)
            nc.scalar.activation(row[:], psr[:1, :], Ln, bias=epsk_t[:1, :], scale=1.0 / P)
        nc.scalar.activation(rs[:], row[:], Exp, scale=-0.5)
        rsb = rsb_pool.tile([P, BLK], F32, name="rsb", tag="rsb")
        nc.gpsimd.partition_broadcast(rsb[:], rs[:])
        rope_tile(dst[:, cols], cols, rsb)

    def proj_block(nb):
        if nb + 1 < NBLK:
            dma_block(nb + 1)
        if nb == 0:
            # k-outer: DMA-paced warmup; uses 6 psum banks across pools
            psms = [scp.tile([P, BLK], F32, name="ps", tag="sc") for _ in range(3)]
            psms.append(attps.tile([P, BLK], F32, name="ps", tag="attps"))
            psms.append(sumps.tile([P, BLK], F32, name="ps", tag="sumps"))
            psms.append(pp.tile([P, BLK], F32, name="ps", tag="pp"))
            for k in range(16):
                for m in range(6):
                    nc.tensor.matmul(
                        psms[m][:], wqkv_sb[k][:, m * P:(m + 1) * P], xts[0][k][:],
                        start=(k == 0), stop=(k == 15), skip_group_check=True,
                    )
            epilogue(0, 5, psms[5])  # v first: frees its pp slot early
            for m in range(5):
                epilogue(0, m, psms[m])
        else:
            for m in range(6):
                ps = pp.tile([P, BLK], F32, name="ps", tag="pp")
                for k in range(16):
                    nc.tensor.matmul(
                        ps[:], wqkv_sb[k][:, m * P:(m + 1) * P], xts[nb][k][:],
                        start=(k == 0), stop=(k == 15), skip_group_check=True,
                    )
                epilogue(nb, m, ps)
        if nb == 1:
            for h in range(HPC):
                nc.sync.dma_start(wo_sb[h][:], wo[h])

    # ---- phase 2: attention (software-pipelined) + Wo per query block ----
    def attn_head(h, qt):
        nkc = 4 * qt + 4
        q0 = qt * BLK
        ps_att = attps.tile([P, BLK], F32, name="psA", tag="attps")
        ps_sum = sumps.tile([P, BLK], F32, name="psB", tag="sumps")

        def scores(kc):
            off = max(0, P * kc - q0)
            ps_s = scp.tile([P, BLK], F32, name="psS", tag="sc")
            nc.tensor.matmul(
                ps_s[:, off:], kT[:, kc * P:(kc + 1) * P], qT[h][:, q0 + off:q0 + BLK],
                start=True, stop=(kc < 4 * qt), skip_group_check=True,
            )
            if kc >= 4 * qt:  # diagonal block: add -30000 upper triangle
                nc.tensor.matmul(
                    ps_s[:, off:off + P], ident[:], mask_t[:],
                    start=False, stop=True, skip_group_check=True,
                )
            ex = exp_pool.tile([P, BLK], BF, name="ex", tag="ex")
            nc.scalar.activation(ex[:, off:], ps_s[:, off:], Exp)
            return kc, off, ex

        def av(kc, off, ex):
            nc.tensor.matmul(
                ps_att[:, off:], v_kd[:, kc * P:(kc + 1) * P], ex[:, off:],
                start=(kc == 0), stop=(kc == nkc - 1), skip_group_check=True,
            )
            nc.tensor.matmul(
                ps_sum[:1, off:], ones_bf[:], ex[:, off:],
                start=(kc == 0), stop=(kc == nkc - 1), skip_group_check=True,
            )

        LAG = 2
        pend = []
        for kc in range(nkc):
            pend.append(scores(kc))
            if len(pend) > LAG:
                av(*pend.pop(0))
        while pend:
            av(*pend.pop(0))

        rrow = row_pool.tile([1, BLK], F32, name="rrow", tag="row")
        nc.vector.reciprocal_approx_fast(rrow[:], ps_sum[:1, :])
        rsb = rsb_pool.tile([P, BLK], F32, name="rsbn", tag="rsb")
        nc.gpsimd.partition_broadcast(rsb[:], rrow[:])
        att_un = attu_pool.tile([P, BLK], BF, name="attu", tag="attu")
        nc.scalar.activation(att_un[:], ps_att[:], Copy)
        a = att_pool.tile([P, BLK], BF, name=f"att{h}", tag=f"att{h}")
        nc.vector.tensor_tensor(a[:], att_un[:], rsb[:], MULT)
        return a

    def wo_block(qt, atts):
        q0 = qt * BLK
        for tc4 in range(4):
            osb = osb_pool.tile([P, D], BF, name="osb", tag="osb")
            for et in range(4):
                ps = pp.tile([P, 512], F32, name="pso", tag="pp")
                for h2 in range(HPC):
                    nc.tensor.matmul(
                        ps[:], atts[h2][:, tc4 * P:(tc4 + 1) * P],
                        wo_sb[h2][:, et * 512:(et + 1) * 512],
                        start=(h2 == 0), stop=(h2 == HPC - 1), skip_group_check=True,
                    )
                nc.vector.tensor_copy(osb[:, et * 512:(et + 1) * 512], ps[:])
            nc.sync.dma_start(out[q0 + tc4 * P:q0 + (tc4 + 1) * P, :], osb[:])

    for nb in range(NBLK):
        proj_block(nb)
    for qt in range(NBLK):
        atts = [attn_head(h, qt) for h in range(HPC)]
        wo_block(qt, atts)


_NC_CACHE = None


def build_nc():
    global _NC_CACHE
    if _NC_CACHE is not None:
        return _NC_CACHE
    nc = bacc.Bacc(None, target_bir_lowering=False)
    xt = nc.dram_tensor("xt", [16, P, S], BF, kind="ExternalInput")
    wqkv = nc.dram_tensor("wqkv", [16, P, 768], BF, kind="ExternalInput")
    wo = nc.dram_tensor("wo", [HPC, P, D], BF, kind="ExternalInput")
    cos = nc.dram_tensor("cos", [P, S], BF, kind="ExternalInput")
    sins = nc.dram_tensor("sins", [P, S], BF, kind="ExternalInput")
    masktri = nc.dram_tensor("masktri", [P, P], BF, kind="ExternalInput")
    gq = nc.dram_tensor("gq", [P, 1], F32, kind="ExternalInput")
    gk = nc.dram_tensor("gk", [P, 1], F32, kind="ExternalInput")
    out = nc.dram_tensor("out", [S, D], BF, kind="ExternalOutput")
    with tile.TileContext(nc) as tc:
        with ExitStack() as ctx:
            _body(ctx, tc, xt[:], wqkv[:], wo[:], cos[:], sins[:], masktri[:],
                  gq[:], gk[:], out[:])
    nc.compile()
    _NC_CACHE = nc
    return nc


def _host_tables():
    pos = np.arange(S, dtype=np.float64)
    inv_freq = 1.0 / (ROPE_THETA ** (np.arange(0, DH, 2, dtype=np.float64) / DH))
    ang = pos[:, None] * inv_freq[None, :]  # [S, 64]
    cos_s = np.concatenate([np.cos(ang), np.cos(ang)], axis=-1)  # [S, 128]
    sin_s = np.concatenate([np.sin(ang), np.sin(ang)], axis=-1)
    cos_full = np.ascontiguousarray(cos_s.T).astype(BFNP)  # [128, S]
    sins = sin_s.T.copy()
    sins[0:64] *= -1.0  # rotation sign baked in
    sins = np.ascontiguousarray(sins).astype(BFNP)
    j = np.arange(P)[:, None]
    i = np.arange(P)[None, :]
    masktri = np.where(j <= i, 0.0, -30000.0).astype(BFNP)  # [keys, queries]
    return cos_full, sins, masktri


def kernel(qkv, Wq, Wk, Wv, Wo, q_gamma, k_gamma):
    qkv = np.asarray(qkv, dtype=np.float32)
    Wq = np.asarray(Wq, dtype=np.float32)
    Wk = np.asarray(Wk, dtype=np.float32)
    Wv = np.asarray(Wv, dtype=np.float32)
    Wo = np.asarray(Wo, dtype=np.float32)
    q_gamma = np.asarray(q_gamma, dtype=np.float32)
    k_gamma = np.asarray(k_gamma, dtype=np.float32)

    nc = build_nc()
    cos_full, sins, masktri = _host_tables()
    gq = np.ascontiguousarray(q_gamma.reshape(P, 1))
    gk = np.ascontiguousarray(k_gamma.reshape(P, 1))
    xts = [np.ascontiguousarray(qkv[b].T).astype(BFNP).reshape(16, P, S) for b in range(B)]

    in_maps = []
    for c in range(NCORES):
        b, g = c // 4, c % 4
        wq_c = Wq[4 * g * DH:(4 * g + 4) * DH, :]  # [512, D]
        wk_c = Wk[g * DH:(g + 1) * DH, :]  # [128, D]
        wv_c = Wv[g * DH:(g + 1) * DH, :]
        wqkv_c = np.concatenate([wq_c, wk_c, wv_c], axis=0).T  # [D, 768]
        wqkv_c = np.ascontiguousarray(wqkv_c).astype(BFNP).reshape(16, P, 768)
        wo_c = np.stack(
            [np.ascontiguousarray(Wo[:, (4 * g + h) * DH:(4 * g + h + 1) * DH].T)
             for h in range(HPC)]
        ).astype(BFNP)  # [4, 128, D]
        in_maps.append({
            "xt": xts[b], "wqkv": wqkv_c, "wo": wo_c,
            "cos": cos_full, "sins": sins, "masktri": masktri,
            "gq": gq, "gk": gk,
        })

    res = run_bass_kernel_spmd(nc, in_maps, core_ids=list(range(NCORES)))
    full = np.empty((B, S, D), np.float32)
    for b in range(B):
        acc = res.results[4 * b]["out"].astype(np.float32)
        for g in range(1, 4):
            acc += res.results[4 * b + g]["out"].astype(np.float32)
        full[b] = acc
    return full
